# revision 1
# baseline (speedup 1.0000x reference)
"""RNN-T JointNetwork kernel for 8 Trainium2 NeuronCores.

reference:
    combined = f[:, :, None, :] + p[:, None, :, :]   # (B,T,U,H)
    h = relu(combined)
    logits = einsum('btuh,vh->btuv', h, W) + b        # (B,T,U,V)

Shapes: f (8,256,640) p (8,64,640) W (1024,640) b (1024,) -> out (8,256,64,1024) f32.

Sharding: data-parallel over B — core i computes batch i. W/b replicated.

Per-core program (SPMD, f32r matmuls):
  - inputs pre-transposed on host: ft=f[b].T (640,256), pt=p[b].T (640,64),
    wt=W.T (640,1024), bias replicated to (128,1024).
  - h_u[h,t] = relu(ft[h,t] + pt[h,u]) via ScalarE activation (bias = pt column).
  - logits[t, u, :] via PE: out[tile] = h_u[kchunk, tslice].T @ wt[kchunk, vslice]
    accumulated over 5 k-chunks into PSUM; DVE adds bias while copying PSUM->SBUF;
    staged SBUF tiles are DMA'd out 2 MiB at a time ((128 t) x (4 u) x (1024 v)).
"""

import numpy as np

import concourse.bass as bass
import concourse.mybir as mybir
import concourse.tile as tile
from concourse.bass_utils import run_bass_kernel_spmd
from concourse.vector_clock import ScopedClock

B, T, U, H, V = 8, 256, 64, 640, 1024
KC = H // 128          # 5 contraction chunks
TC = T // 128          # 2 t chunks
N_CORES = 8
UG = 4                 # u values staged per output DMA (2 MiB per DMA)
MM_DT = mybir.dt.float32r

_PATCHED = False


_MAX_WAITS = 1  # this walrus build rejects >1 sem-wait per instruction


def _spill_waits(nc, inst, add):
    """If `inst` carries more than _MAX_WAITS sem-waits, move the excess onto
    same-engine nops emitted (in program order) just before it."""
    si = inst.sync_info
    waits = list(si.on_wait) if si and si.on_wait else []
    if len(waits) <= _MAX_WAITS:
        return
    excess = waits[: len(waits) - _MAX_WAITS]
    inst.sync_info = mybir.SyncInfo(
        on_wait=waits[len(waits) - _MAX_WAITS :],
        on_update=list(si.on_update or []),
    )
    for i in range(0, len(excess), _MAX_WAITS):
        nop = mybir.InstNoOp(name=f"{inst.name}_spillw{i}", ins=[], outs=[])
        nop.engine = inst.engine
        nop.sync_info = mybir.SyncInfo(
            on_wait=excess[i : i + _MAX_WAITS], on_update=[]
        )
        nc.register_instruction(nop, overwrite=True)
        add(nop)


def _patch_tile_drain():
    """This walrus build's setupSyncWait rejects instructions carrying more
    than one sem-wait.  Tile freely emits several per instruction, so (a)
    split excess waits onto same-engine nops as instructions are committed
    into basic blocks, and (b) do the same for the end-of-kernel drain."""
    global _PATCHED
    if _PATCHED:
        return
    _PATCHED = True

    orig_add = tile.TileContext._add_instruction

    def _add_instruction(self, inst):
        _spill_waits(self.nc, inst, lambda n: orig_add(self, n))
        orig_add(self, inst)

    tile.TileContext._add_instruction = _add_instruction

    def _drain_and_barrier(self, tick_clock, wait_clock):
        nc = self.nc
        probe = nc.sync.nop(nofuse=True, hint="drain_wait_probe")
        wait_clock.add_sem_waits(
            probe.ins, ScopedClock({None: tick_clock.global_clock})
        )
        si = probe.ins.sync_info
        waits = list(si.on_wait) if si and si.on_wait else []
        if len(waits) > _MAX_WAITS:
            probe.ins.sync_info = mybir.SyncInfo(
                on_wait=waits[:_MAX_WAITS], on_update=list(si.on_update or [])
            )
            rest = waits[_MAX_WAITS:]
            for i in range(0, len(rest), _MAX_WAITS):
                extra = nc.sync.nop(nofuse=True, hint=f"drain_wait_{i}")
                extra.ins.sync_info = mybir.SyncInfo(
                    on_wait=rest[i : i + _MAX_WAITS], on_update=[]
                )
        nc.sync.drain()
        nc.all_engine_barrier()
        assert self.sems is not None
        popped = nc._tile_sem_poison_stack.pop()
        assert popped is self._sem_poison
        nc.clear_and_free_semaphores(list(self.sems.allocated().values()))
        nc.all_engine_barrier()

    tile.TileContext._drain_and_barrier = _drain_and_barrier


def build_program():
    """One SPMD NeuronCore program: (T,U,V) joint-network slice for one batch."""
    _patch_tile_drain()
    nc = bass.Bass()
    f32 = mybir.dt.float32

    ft = nc.dram_tensor("ft", [H, T], f32, kind="ExternalInput")
    pt = nc.dram_tensor("pt", [H, U], f32, kind="ExternalInput")
    wt = nc.dram_tensor("wt", [H, V], MM_DT, kind="ExternalInput")
    bias = nc.dram_tensor("bias", [128, V], f32, kind="ExternalInput")
    out = nc.dram_tensor("out", [T, U, V], f32, kind="ExternalOutput")

    ft_v = ft.rearrange("(k p) t -> p k t", p=128)
    pt_v = pt.rearrange("(k p) u -> p k u", p=128)
    wt_v = wt.rearrange("(k p) v -> p k v", p=128)

    with tile.TileContext(nc) as tc:
        with (
            tc.tile_pool(name="const", bufs=1) as cpool,
            tc.tile_pool(name="h", bufs=3) as hpool,
            tc.tile_pool(name="stage", bufs=3) as spool,
            tc.tile_pool(name="psum", bufs=8, space="PSUM") as ppool,
        ):
            ft_sb = cpool.tile([128, KC, T], f32)
            pt_sb = cpool.tile([128, KC, U], f32)
            wt_ks = [cpool.tile([128, V], MM_DT, name=f"wt_k{k}")
                     for k in range(KC)]
            bias_sb = cpool.tile([128, V], f32)
            nc.sync.dma_start(ft_sb[:], ft_v[:])
            nc.sync.dma_start(pt_sb[:], pt_v[:])
            for k in range(KC):
                nc.sync.dma_start(wt_ks[k][:], wt_v[:, k, :])
            nc.sync.dma_start(bias_sb[:], bias[:])


            for u0 in range(0, U, UG):
                stages = [spool.tile([128, UG, V], f32, tag=f"st{t_}",
                                     name=f"stage{t_}_{u0}")
                          for t_ in range(TC)]
                for j in range(UG):
                    u = u0 + j
                    h_u = hpool.tile([128, KC, T], MM_DT, tag="h")
                    for k in range(KC):
                        nc.scalar.activation(
                            h_u[:, k, :],
                            ft_sb[:, k, :],
                            mybir.ActivationFunctionType.Relu,
                            bias=pt_sb[:, k, u : u + 1],
                        )
                    for t_ in range(TC):
                        psums = [ppool.tile([128, 512], f32, tag="ps",
                                            name=f"ps{u}_{t_}_{h_}")
                                 for h_ in range(2)]
                        for k in range(KC):
                            lhsT = h_u[:, k, t_ * 128 : (t_ + 1) * 128]
                            for h_ in range(2):
                                nc.tensor.matmul(
                                    psums[h_][:],
                                    lhsT,
                                    wt_ks[k][:, h_ * 512 : (h_ + 1) * 512],
                                    start=(k == 0),
                                    stop=(k == KC - 1),
                                )
                        for h_ in range(2):
                            sl = slice(h_ * 512, (h_ + 1) * 512)
                            nc.vector.tensor_add(
                                stages[t_][:, j, sl],
                                psums[h_][:],
                                bias_sb[:, sl],
                            )
                for t_ in range(TC):
                    nc.sync.dma_start(
                        out[t_ * 128 : (t_ + 1) * 128, u0 : u0 + UG, :],
                        stages[t_][:],
                    )
    return nc


def kernel(f, p, W, b):
    f = np.asarray(f, np.float32)
    p = np.asarray(p, np.float32)
    W = np.asarray(W, np.float32)
    b = np.asarray(b, np.float32)

    nc = build_program()

    wt = np.ascontiguousarray(W.T)                      # (H, V)
    bias = np.ascontiguousarray(np.broadcast_to(b, (128, V)))
    in_maps = [
        {
            "ft": np.ascontiguousarray(f[i].T),         # (H, T)
            "pt": np.ascontiguousarray(p[i].T),         # (H, U)
            "wt": wt,
            "bias": bias,
        }
        for i in range(N_CORES)
    ]
    res = run_bass_kernel_spmd(nc, in_maps, list(range(N_CORES)))
    return np.stack([res.results[i]["out"] for i in range(N_CORES)], axis=0)



# revision 2
# speedup vs baseline: 1.0885x; 1.0885x over previous
"""RNN-T JointNetwork kernel for 8 Trainium2 NeuronCores.

reference:
    combined = f[:, :, None, :] + p[:, None, :, :]   # (B,T,U,H)
    h = relu(combined)
    logits = einsum('btuh,vh->btuv', h, W) + b        # (B,T,U,V)

Shapes: f (8,256,640) p (8,64,640) W (1024,640) b (1024,) -> out (8,256,64,1024) f32.

Sharding: data-parallel over B — core i computes batch i. W/b replicated.

Per-core program (SPMD, bf16 matmuls, rel-err budget 2e-2; bf16 end-to-end
measures ~3e-3):
  - inputs pre-transposed + cast to bf16 on host: ft=f[b].T (640,256),
    pt=p[b].T (640,64), wt=W.T (640,1024), bias replicated to (128,1024).
  - PE warm-up: a dozen throwaway matmuls on the (tiny, first-landed) pt tile
    run during the input-DMA window so the HAM clock gate is at 8/8 before the
    first real matmul.
  - h_u[h,t] = relu(ft[h,t] + pt[h,u]) via ScalarE activation (bias = pt
    column), output bf16.
  - logits[t, u, :] via PE: out[tile] = h_u[kchunk, tslice].T @ wt[kchunk,
    vslice] accumulated over 5 k-chunks into PSUM; DVE adds bias while copying
    PSUM->SBUF as bf16; staged tiles are DMA'd out 1 MiB at a time, per-u
    (256 KiB) for the last group to shorten the tail.
  - host casts the bf16 output back to f32.
"""

import ml_dtypes
import numpy as np

import concourse.bass as bass
import concourse.mybir as mybir
import concourse.tile as tile
from concourse.bass_utils import run_bass_kernel_spmd
from concourse.vector_clock import ScopedClock

B, T, U, H, V = 8, 256, 64, 640, 1024
KC = H // 128          # 5 contraction chunks
TC = T // 128          # 2 t chunks
N_CORES = 8
UG = 4                 # u values staged per output DMA
N_WARMUP_MM = 14       # ~3.5us of cold-rate matmuls to open the HAM clock gate

_PATCHED = False


_MAX_WAITS = 1  # this walrus build rejects >1 sem-wait per instruction


def _spill_waits(nc, inst, add):
    """If `inst` carries more than _MAX_WAITS sem-waits, move the excess onto
    same-engine nops emitted (in program order) just before it."""
    si = inst.sync_info
    waits = list(si.on_wait) if si and si.on_wait else []
    if len(waits) <= _MAX_WAITS:
        return
    excess = waits[: len(waits) - _MAX_WAITS]
    inst.sync_info = mybir.SyncInfo(
        on_wait=waits[len(waits) - _MAX_WAITS :],
        on_update=list(si.on_update or []),
    )
    for i in range(0, len(excess), _MAX_WAITS):
        nop = mybir.InstNoOp(name=f"{inst.name}_spillw{i}", ins=[], outs=[])
        nop.engine = inst.engine
        nop.sync_info = mybir.SyncInfo(
            on_wait=excess[i : i + _MAX_WAITS], on_update=[]
        )
        nc.register_instruction(nop, overwrite=True)
        add(nop)


def _patch_tile_drain():
    """This walrus build's setupSyncWait rejects instructions carrying more
    than one sem-wait.  Tile freely emits several per instruction, so (a)
    split excess waits onto same-engine nops as instructions are committed
    into basic blocks, and (b) do the same for the end-of-kernel drain."""
    global _PATCHED
    if _PATCHED:
        return
    _PATCHED = True

    orig_add = tile.TileContext._add_instruction

    def _add_instruction(self, inst):
        _spill_waits(self.nc, inst, lambda n: orig_add(self, n))
        orig_add(self, inst)

    tile.TileContext._add_instruction = _add_instruction

    def _drain_and_barrier(self, tick_clock, wait_clock):
        nc = self.nc
        probe = nc.sync.nop(nofuse=True, hint="drain_wait_probe")
        wait_clock.add_sem_waits(
            probe.ins, ScopedClock({None: tick_clock.global_clock})
        )
        si = probe.ins.sync_info
        waits = list(si.on_wait) if si and si.on_wait else []
        if len(waits) > _MAX_WAITS:
            probe.ins.sync_info = mybir.SyncInfo(
                on_wait=waits[:_MAX_WAITS], on_update=list(si.on_update or [])
            )
            rest = waits[_MAX_WAITS:]
            for i in range(0, len(rest), _MAX_WAITS):
                extra = nc.sync.nop(nofuse=True, hint=f"drain_wait_{i}")
                extra.ins.sync_info = mybir.SyncInfo(
                    on_wait=rest[i : i + _MAX_WAITS], on_update=[]
                )
        nc.sync.drain()
        nc.all_engine_barrier()
        assert self.sems is not None
        popped = nc._tile_sem_poison_stack.pop()
        assert popped is self._sem_poison
        nc.clear_and_free_semaphores(list(self.sems.allocated().values()))
        nc.all_engine_barrier()

    tile.TileContext._drain_and_barrier = _drain_and_barrier


def build_program():
    """One SPMD NeuronCore program: (T,U,V) joint-network slice for one batch."""
    _patch_tile_drain()
    nc = bass.Bass()
    f32 = mybir.dt.float32
    bf16 = mybir.dt.bfloat16

    ft = nc.dram_tensor("ft", [H, T], bf16, kind="ExternalInput")
    pt = nc.dram_tensor("pt", [H, U], bf16, kind="ExternalInput")
    wt = nc.dram_tensor("wt", [H, V], bf16, kind="ExternalInput")
    bias = nc.dram_tensor("bias", [128, V], bf16, kind="ExternalInput")
    out = nc.dram_tensor("out", [T, U, V], bf16, kind="ExternalOutput")

    ft_v = ft.rearrange("(k p) t -> p k t", p=128)
    pt_v = pt.rearrange("(k p) u -> p k u", p=128)
    wt_v = wt.rearrange("(k p) v -> p k v", p=128)

    with tile.TileContext(nc) as tc:
        with (
            tc.tile_pool(name="const", bufs=1) as cpool,
            tc.tile_pool(name="h", bufs=3) as hpool,
            tc.tile_pool(name="stage", bufs=3) as spool,
            tc.tile_pool(name="psum", bufs=8, space="PSUM") as ppool,
        ):
            ft_sb = cpool.tile([128, KC, T], bf16)
            pt_sb = cpool.tile([128, KC, U], bf16)
            wt_ks = [cpool.tile([128, V], bf16, name=f"wt_k{k}")
                     for k in range(KC)]
            bias_sb = cpool.tile([128, V], bf16)
            # pt first: it is tiny and feeds both the PE warm-up and the
            # activations; then ft + the first weight chunk (the real-matmul
            # gate), then the rest.
            nc.sync.dma_start(pt_sb[:], pt_v[:])
            nc.sync.dma_start(ft_sb[:], ft_v[:])
            for k in range(KC):
                nc.sync.dma_start(wt_ks[k][:], wt_v[:, k, :])
            nc.sync.dma_start(bias_sb[:], bias[:])

            # PE warm-up on the pt tile (values irrelevant, result unread):
            # keeps the PE busy through the HAM activity window while the
            # remaining input DMAs land, so real matmuls start at 2.4 GHz.
            warm_ps = ppool.tile([64, 320], f32, tag="ps", name="warm_ps")
            for w in range(N_WARMUP_MM):
                nc.tensor.matmul(
                    warm_ps[:],
                    pt_sb[:, 0, :],
                    pt_sb[:, :, :],
                    start=True,
                    stop=True,
                )

            for u0 in range(0, U, UG):
                last_group = u0 + UG >= U
                stages = [spool.tile([128, UG, V], bf16, tag=f"st{t_}",
                                     name=f"stage{t_}_{u0}")
                          for t_ in range(TC)]
                for j in range(UG):
                    u = u0 + j
                    h_u = hpool.tile([128, KC, T], bf16, tag="h")
                    for k in range(KC):
                        nc.scalar.activation(
                            h_u[:, k, :],
                            ft_sb[:, k, :],
                            mybir.ActivationFunctionType.Relu,
                            bias=pt_sb[:, k, u : u + 1],
                        )
                    for t_ in range(TC):
                        psums = [ppool.tile([128, 512], f32, tag="ps",
                                            name=f"ps{u}_{t_}_{h_}")
                                 for h_ in range(2)]
                        for k in range(KC):
                            lhsT = h_u[:, k, t_ * 128 : (t_ + 1) * 128]
                            for h_ in range(2):
                                nc.tensor.matmul(
                                    psums[h_][:],
                                    lhsT,
                                    wt_ks[k][:, h_ * 512 : (h_ + 1) * 512],
                                    start=(k == 0),
                                    stop=(k == KC - 1),
                                )
                        for h_ in range(2):
                            sl = slice(h_ * 512, (h_ + 1) * 512)
                            nc.vector.tensor_add(
                                stages[t_][:, j, sl],
                                psums[h_][:],
                                bias_sb[:, sl],
                            )
                    if last_group:
                        # per-u output DMA at the end: the tail after the
                        # final matmul only has to drain 256 KiB, not 1 MiB
                        for t_ in range(TC):
                            nc.sync.dma_start(
                                out[t_ * 128 : (t_ + 1) * 128,
                                    u : u + 1, :],
                                stages[t_][:, j : j + 1, :],
                            )
                if not last_group:
                    for t_ in range(TC):
                        nc.sync.dma_start(
                            out[t_ * 128 : (t_ + 1) * 128, u0 : u0 + UG, :],
                            stages[t_][:],
                        )
    return nc


def prepare_inputs(f, p, W, b):
    """Host-side shard + layout prep: per-core bf16 in_maps."""
    f = np.asarray(f, np.float32)
    p = np.asarray(p, np.float32)
    W = np.asarray(W, np.float32)
    b = np.asarray(b, np.float32)
    bf = ml_dtypes.bfloat16
    wt = np.ascontiguousarray(W.T).astype(bf)                   # (H, V)
    bias = np.ascontiguousarray(np.broadcast_to(b, (128, V))).astype(bf)
    return [
        {
            "ft": np.ascontiguousarray(f[i].T).astype(bf),      # (H, T)
            "pt": np.ascontiguousarray(p[i].T).astype(bf),      # (H, U)
            "wt": wt,
            "bias": bias,
        }
        for i in range(N_CORES)
    ]


def kernel(f, p, W, b):
    nc = build_program()
    in_maps = prepare_inputs(f, p, W, b)
    res = run_bass_kernel_spmd(nc, in_maps, list(range(N_CORES)))
    out = np.stack([res.results[i]["out"] for i in range(N_CORES)], axis=0)
    return out.astype(np.float32)
